# revision 1
# baseline (speedup 1.0000x reference)
"""Trainium2 Bass kernel for single-head base attention.

Problem: x [4, 2048, 1024] fp32; Wq/Wk/Wv [1024, 1024] (torch [out, in]).
  Q = x @ Wq.T ; K = x @ Wk.T ; V = x @ Wv.T
  out = softmax(Q K^T / 32) V

Sharding: 8 cores = 4 batches x 2 query-halves. Each core computes K/V for
its batch's full 2048-seq and Q for its 1024-query half; outputs are
disjoint [1024, 1024] slices, so no collectives. (A pairwise-AllGather
K/V-dedup variant was tried and measured SLOWER: 2-rank collective_compute
moves ~34 GB/s, so the 8MB exchange costs ~2x the 55us of PE it saves.)

Per-core schedule (projection matmuls float32r; K^T/Q^T/V/attT stored
bf16 — same PE rate, half the SBUF — HW rel err 4.07e-3 vs 2e-2 gate):
  phase K: KT[e,k] = WkT.T @ xT      -> resident SBUF (8MB)
  phase V: V[k,e]  = xT.T @ WvT      -> resident SBUF (8MB)
  phase Q: QT[e,q] = WqT.T @ xqT     -> resident SBUF (4MB)
  attention per 512-query block, S computed TRANSPOSED so exp output is
  directly the stationary operand of the O matmul (no PE transposes, no
  DVE shuffle copies):
      ST[k,q] chunk = KT_slice.T @ QT    per k-tile  [128, 512] PSUM
      attT[k,q]     = exp(ST/32) on ACT  -> SBUF
      O[q,e]        = attT.T @ V         (PSUM accum over k, 2x512 chunks)
      rowsum[q]     = attT.T @ ones      (third 8-wide PSUM group, ~free)
      out = O * (1/rowsum)               -> DRAM
"""

import os
import sys
from contextlib import ExitStack

import numpy as np

for _p in ("/opt/trn_rl_repo", "/root/.axon_site/_ro/trn_rl_repo"):
    if os.path.isdir(_p) and _p not in sys.path:
        sys.path.append(_p)

import concourse.bass as bass
import concourse.mybir as mybir
from concourse import bacc, tile
from concourse.bass_utils import run_bass_kernel_spmd

F32 = mybir.dt.float32
F32R = mybir.dt.float32r  # tf32-rate matmul dtype, fp32 storage bits
BF16 = mybir.dt.bfloat16   # K/Q/V + attT storage: same PE rate, half SBUF

B, SEQ, D = 4, 2048, 1024
QL = SEQ // 2          # queries per core
N_CORES = 8
DT = D // 128          # 8 d-tiles (contraction)
ET = D // 128          # 8 e-tiles (hidden out)
KT = SEQ // 128        # 16 k-tiles
QT = QL // 128         # 8 q-tiles per core
XB = 256               # xT streaming col-block width
NXB = SEQ // XB        # 8 blocks
QB = 512               # attention q-block width
NQB = QL // QB         # 2 q-blocks
AF = mybir.ActivationFunctionType


def _copy(nc, i, dst, src):
    # alternate PSUM->SBUF copies between DVE and ACT to balance engines
    if i % 2 == 0:
        nc.vector.tensor_copy(dst, src)
    else:
        nc.scalar.copy(dst, src)


def build(reps: int = 1):
    nc = bacc.Bacc(
        "TRN2", target_bir_lowering=False, debug=False, num_devices=N_CORES
    )

    xT = nc.declare_dram_parameter("xT", [D, SEQ], F32R, isOutput=False)
    xqT = nc.declare_dram_parameter("xqT", [D, QL], F32R, isOutput=False)
    WqT = nc.declare_dram_parameter("WqT", [D, D], F32R, isOutput=False)
    WkT = nc.declare_dram_parameter("WkT", [D, D], F32R, isOutput=False)
    WvT = nc.declare_dram_parameter("WvT", [D, D], F32R, isOutput=False)
    out = nc.declare_dram_parameter("out", [QL, D], F32, isOutput=True)

    aps = {
        "xT_r": xT.rearrange("(dt p) k -> p dt k", p=128),
        "xqT_r": xqT.rearrange("(dt p) q -> p dt q", p=128),
        "Wq_r": WqT.rearrange("(dt p) e -> p dt e", p=128),
        "Wk_r": WkT.rearrange("(dt p) e -> p dt e", p=128),
        "Wv_r": WvT.rearrange("(dt p) e -> p dt e", p=128),
        "out_r": out.rearrange("(qt p) e -> qt p e", p=128),
    }

    with ExitStack() as top:
        tc = top.enter_context(tile.TileContext(nc))

        res_pool = top.enter_context(tc.tile_pool(name="res", bufs=1))

        kt_sb = res_pool.tile([128, ET, SEQ], BF16, tag="kt_sb")
        v_sb = res_pool.tile([128, KT, D], BF16, tag="v_sb")

        for _ in range(reps):
            _body(nc, tc, kt_sb, v_sb, aps)

    nc.compile()
    return nc


def _body(nc, tc, kt_sb, v_sb, aps):
    xT_r = aps["xT_r"]
    xqT_r = aps["xqT_r"]
    Wq_r = aps["Wq_r"]
    Wk_r = aps["Wk_r"]
    Wv_r = aps["Wv_r"]
    out_r = aps["out_r"]

    # phv_w outlives phase K so wv can prefetch during K's matmuls
    # (SBUF cap is ~208KB/partition; phase K peaks at ~208.2).
    with tc.tile_pool(name="phv_w", bufs=1) as phv_w:
        # wv split 7/1: the 28KB head prefetches during phase K, the 4KB
        # tail loads at the start of phase V
        wv_a = phv_w.tile([128, 7, D], F32R, tag="wv_a")

        # ---------------- phase K: KT[e,k] resident ----------------
        ph_x = tc.tile_pool(name="ph_x", bufs=3)
        ph_x_pool = ph_x.__enter__()
        last_xtb = None
        with (
            tc.tile_pool(name="phk_w", bufs=1) as phk_w,
            tc.tile_pool(name="psk", bufs=4, space="PSUM") as psk,
        ):
            # wk in two halves; wk_a additionally split into per-d-row DMAs
            # so the first warm-up matmul starts after ~1.5MB of DMA
            wk_a = phk_w.tile([128, 4, D], F32R, tag="wk_a")
            wk_b = phk_w.tile([128, 4, D], F32R, tag="wk_b")

            def wkd(d, et):
                half = wk_a if d < 4 else wk_b
                return half[:, d % 4, et * 128 : (et + 1) * 128]

            ci = 0
            for j in range(NXB):
                xtb = ph_x_pool.tile([128, DT, XB], F32R, tag="xtb")
                nc.sync.dma_start(xtb[:], xT_r[:, :, j * XB : (j + 1) * XB])
                if j == NXB - 1:
                    last_xtb = xtb
                if j == 0:
                    nc.sync.dma_start(wk_a[:], Wk_r[:, :4, :])
                    nc.sync.dma_start(wk_b[:], Wk_r[:, 4:, :])
                if j == 2:
                    # ~40us of K matmuls remain: hides the 3.5MB wv_a load
                    nc.sync.dma_start(wv_a[:], Wv_r[:, :7, :])
                if j == 0:
                    # split-accumulation warm-up: run d 0-3 of the first four
                    # groups on wk_a alone, then finish with wk_b
                    pss0 = []
                    for et in range(4):
                        ps = psk.tile([128, XB], F32, tag="ps")
                        for d in range(4):
                            nc.tensor.matmul(
                                ps[:], wkd(d, et), xtb[:, d, :],
                                start=(d == 0), stop=False,
                            )
                        pss0.append(ps)
                    for et in range(4):
                        ps = pss0[et]
                        for d in range(4, DT):
                            nc.tensor.matmul(
                                ps[:], wkd(d, et), xtb[:, d, :],
                                start=False, stop=(d == DT - 1),
                            )
                        _copy(nc, ci, kt_sb[:, et, j * XB : (j + 1) * XB], ps[:])
                        ci += 1
                    rest = range(4, ET)
                else:
                    rest = range(ET)
                for et in rest:
                    ps = psk.tile([128, XB], F32, tag="ps")
                    for d in range(DT):
                        nc.tensor.matmul(
                            ps[:], wkd(d, et), xtb[:, d, :],
                            start=(d == 0), stop=(d == DT - 1),
                        )
                    _copy(nc, ci, kt_sb[:, et, j * XB : (j + 1) * XB], ps[:])
                    ci += 1


        # ---------------- phase V: V[k,e] resident ----------------
        with (
            tc.tile_pool(name="phv_wb", bufs=1) as phv_wb,
            tc.tile_pool(name="psv", bufs=4, space="PSUM") as psv,
        ):
            wv_b = phv_wb.tile([128, 1, D], F32R, tag="wv_b")
            nc.sync.dma_start(wv_b[:], Wv_r[:, 7:, :])
            ci = 0
            # reverse order: block NXB-1 is still resident from phase K
            for j in range(NXB - 1, -1, -1):
                if j == NXB - 1 and last_xtb is not None:
                    xtb = last_xtb
                else:
                    xtb = ph_x_pool.tile([128, DT, XB], F32R, tag="xtb")
                    nc.sync.dma_start(xtb[:], xT_r[:, :, j * XB : (j + 1) * XB])
                for k2 in range(XB // 128):
                    kt = j * (XB // 128) + k2
                    for ec in range(D // 512):
                        ps = psv.tile([128, 512], F32, tag="ps")
                        for d in range(DT):
                            wvd = (
                                wv_a[:, d, ec * 512 : (ec + 1) * 512]
                                if d < 7
                                else wv_b[:, d - 7, ec * 512 : (ec + 1) * 512]
                            )
                            nc.tensor.matmul(
                                ps[:],
                                xtb[:, d, k2 * 128 : (k2 + 1) * 128],
                                wvd,
                                start=(d == 0),
                                stop=(d == DT - 1),
                            )
                        _copy(nc, ci, v_sb[:, kt, ec * 512 : (ec + 1) * 512], ps[:])
                        ci += 1
        ph_x.__exit__(None, None, None)

    # ---------------- phase Q: QT[e,q] resident ----------------
    # wq streamed per e-column (0.5MB x3 bufs) and xq per 512-query block.
    # Both xq and wq DMAs are split per-d-tile so the first Q matmuls start
    # ~1.5us after phase V's last xtb read frees the space, not ~7.5us.
    with tc.tile_pool(name="res2", bufs=1) as res2:
        qt_sb = res2.tile([128, ET, QL], BF16, tag="qt_sb")
        with (
            tc.tile_pool(name="phq_w", bufs=3) as phq_w,
            tc.tile_pool(name="phq_x", bufs=2) as phq_x,
            tc.tile_pool(name="psq", bufs=4, space="PSUM") as psq,
        ):
            ci = 0
            for qc in range(QL // 512):
                xq = phq_x.tile([128, DT, 512], F32R, tag="xq")
                nc.sync.dma_start(xq[:], xqT_r[:, :, qc * 512 : (qc + 1) * 512])
                for et in range(ET):
                    wqc = phq_w.tile([128, DT, 128], F32R, tag="wqc")
                    nc.sync.dma_start(
                        wqc[:], Wq_r[:, :, et * 128 : (et + 1) * 128]
                    )
                    ps = psq.tile([128, 512], F32, tag="ps")
                    for d in range(DT):
                        nc.tensor.matmul(
                            ps[:],
                            wqc[:, d, :],
                            xq[:, d, :],
                            start=(d == 0),
                            stop=(d == DT - 1),
                        )
                    _copy(nc, ci, qt_sb[:, et, qc * 512 : (qc + 1) * 512], ps[:])
                    ci += 1

        # ---------------- attention (S transposed) ----------------
        with (
            tc.tile_pool(name="attn", bufs=1) as attn_p,
            tc.tile_pool(name="osb_p", bufs=2) as osb_p,
            tc.tile_pool(name="vec_p", bufs=8) as vec_p,
            tc.tile_pool(name="pss", bufs=2, space="PSUM") as pss,
            tc.tile_pool(name="pso", bufs=2, space="PSUM") as pso,
        ):
            attT = attn_p.tile([128, KT, QB], BF16, tag="attT")
            ones_f = attn_p.tile([128, 8], F32, tag="ones_f")
            ones = attn_p.tile([128, 8], BF16, tag="ones")
            nc.vector.memset(ones_f[:], 1.0)
            nc.vector.tensor_copy(ones[:], ones_f[:])

            for qb in range(NQB):
                for kt in range(KT):
                    ps = pss.tile([128, QB], F32, tag="S")
                    for et in range(ET):
                        nc.tensor.matmul(
                            ps[:],
                            kt_sb[:, et, kt * 128 : (kt + 1) * 128],
                            qt_sb[:, et, qb * QB : (qb + 1) * QB],
                            start=(et == 0),
                            stop=(et == ET - 1),
                        )
                    nc.scalar.activation(
                        attT[:, kt, :], ps[:], AF.Exp, scale=1.0 / 32.0
                    )
                for q4 in range(QB // 128):
                    qt = qb * (QB // 128) + q4
                    po0 = pso.tile([128, 512], F32, tag="po0")
                    po1 = pso.tile([128, 512], F32, tag="po1")
                    pos = pso.tile([128, 8], F32, tag="pos")
                    pch = (po0, po1)
                    att_q = attT[:, :, q4 * 128 : (q4 + 1) * 128]
                    for kt in range(KT):
                        st = kt == 0
                        sp = kt == KT - 1
                        for ec in range(2):
                            nc.tensor.matmul(
                                pch[ec][:],
                                att_q[:, kt, :],
                                v_sb[:, kt, ec * 512 : (ec + 1) * 512],
                                start=st,
                                stop=sp,
                            )
                        nc.tensor.matmul(
                            pos[:], att_q[:, kt, :], ones[:], start=st, stop=sp
                        )
                    r = vec_p.tile([128, 1], F32, tag="r")
                    nc.vector.reciprocal(r[:], pos[:, 0:1])
                    osb = osb_p.tile([128, D], F32, tag="osb")
                    for ec in range(2):
                        nc.vector.tensor_scalar_mul(
                            osb[:, ec * 512 : (ec + 1) * 512], pch[ec][:], r[:]
                        )
                    nc.sync.dma_start(out_r[qt], osb[:])


_CACHE: dict = {}


def _get_nc():
    if "nc" not in _CACHE:
        _CACHE["nc"] = build()
    return _CACHE["nc"]


def _get_runner():
    """Cached jitted shard_map executable over the 8 cores."""
    if "runner" in _CACHE:
        return _CACHE["runner"]

    import jax
    from jax.sharding import Mesh, NamedSharding, PartitionSpec
    from jax.experimental.shard_map import shard_map

    from concourse import bass2jax, mybir as _mybir

    nc = _get_nc()
    bass2jax.install_neuronx_cc_hook()

    partition_name = (
        nc.partition_id_tensor.name if nc.partition_id_tensor else None
    )
    in_names = []
    out_names = []
    out_avals = []
    zero_outs = []
    for alloc in nc.m.functions[0].allocations:
        if not isinstance(alloc, _mybir.MemoryLocationSet):
            continue
        if alloc.kind == "ExternalInput":
            if alloc.memorylocations[0].name == partition_name:
                continue
            in_names.append(alloc.memorylocations[0].name)
        elif alloc.kind == "ExternalOutput":
            name = alloc.memorylocations[0].name
            out_names.append(name)
            shape = tuple(alloc.tensor_shape)
            dtype = _mybir.dt.np(alloc.dtype)
            out_avals.append(jax.core.ShapedArray(shape, dtype))
            zero_outs.append(np.zeros(shape, dtype))
    n_params = len(in_names)
    all_in_names = in_names + out_names
    if partition_name is not None:
        all_in_names = all_in_names + [partition_name]

    def _body_fn(*args):
        operands = list(args)
        if partition_name is not None:
            operands.append(bass2jax.partition_id_tensor())
        outs = bass2jax._bass_exec_p.bind(
            *operands,
            out_avals=tuple(out_avals),
            in_names=tuple(all_in_names),
            out_names=tuple(out_names),
            lowering_input_output_aliases=(),
            sim_require_finite=True,
            sim_require_nnan=True,
            nc=nc,
        )
        return tuple(outs)

    devices = jax.devices()[:N_CORES]
    mesh = Mesh(np.asarray(devices), ("core",))
    spec = PartitionSpec("core")
    n_outs = len(out_names)
    sharded = jax.jit(
        shard_map(
            _body_fn,
            mesh=mesh,
            in_specs=(spec,) * (n_params + n_outs),
            out_specs=(spec,) * n_outs,
            check_rep=False,
        ),
        keep_unused=True,
    )
    sharding = NamedSharding(mesh, spec)

    def run(in_maps):
        concat_in = [
            np.concatenate([np.asarray(m[name]) for m in in_maps], axis=0)
            for name in in_names
        ]
        concat_zeros = [
            np.zeros((N_CORES * z.shape[0], *z.shape[1:]), z.dtype)
            for z in zero_outs
        ]
        dev_in = [jax.device_put(a, sharding) for a in concat_in]
        dev_zero = [jax.device_put(a, sharding) for a in concat_zeros]
        out_arrs = sharded(*dev_in, *dev_zero)
        return [
            {
                name: np.asarray(out_arrs[i]).reshape(
                    N_CORES, *out_avals[i].shape
                )[c]
                for i, name in enumerate(out_names)
            }
            for c in range(N_CORES)
        ]

    def run_device(dev_in, dev_zero):
        return sharded(*dev_in, *dev_zero)

    _CACHE["runner"] = (run, run_device, sharding, in_names, zero_outs)
    return _CACHE["runner"]


def _make_in_maps(x, Wq, Wk, Wv):
    x = np.asarray(x, dtype=np.float32)
    wqT = np.ascontiguousarray(np.asarray(Wq, dtype=np.float32).T)
    wkT = np.ascontiguousarray(np.asarray(Wk, dtype=np.float32).T)
    wvT = np.ascontiguousarray(np.asarray(Wv, dtype=np.float32).T)
    in_maps = []
    for c in range(N_CORES):
        b, h = divmod(c, 2)
        xb = np.ascontiguousarray(x[b].T)
        xq = np.ascontiguousarray(x[b, h * QL : (h + 1) * QL].T)
        in_maps.append(
            {
                "xT": xb,
                "xqT": xq,
                "WqT": wqT,
                "WkT": wkT,
                "WvT": wvT,
            }
        )
    return in_maps


def _assemble(results):
    out = np.empty((B, SEQ, D), dtype=np.float32)
    for c in range(N_CORES):
        b, h = divmod(c, 2)
        out[b, h * QL : (h + 1) * QL] = results[c]["out"]
    return out


def run_traced(x, Wq, Wk, Wv, **kw):
    """Run via run_bass_kernel_spmd, return (output, BassKernelResults)."""
    nc = _get_nc()
    res = run_bass_kernel_spmd(
        nc, _make_in_maps(x, Wq, Wk, Wv), list(range(N_CORES)), **kw
    )
    return _assemble(res.results), res


def kernel(x, Wq, Wk, Wv):
    # The axon tunnel occasionally throws transient device errors
    # (worker hung up / mesh desynced / NRT_EXEC_UNIT_UNRECOVERABLE) that
    # clear on retry; rebuild the jitted runner on the second attempt.
    in_maps = _make_in_maps(x, Wq, Wk, Wv)
    last_err = None
    for attempt in range(3):
        try:
            run, _, _, _, _ = _get_runner()
            return _assemble(run(in_maps))
        except Exception as e:  # noqa: BLE001 - retry transient device faults
            last_err = e
            import time as _time

            _time.sleep(5 * (attempt + 1))
            _CACHE.pop("runner", None)
    raise last_err



# revision 17
# speedup vs baseline: 1.1757x; 1.1757x over previous
"""Trainium2 Bass kernel for single-head base attention.

Problem: x [4, 2048, 1024] fp32; Wq/Wk/Wv [1024, 1024] (torch [out, in]).
  Q = x @ Wq.T ; K = x @ Wk.T ; V = x @ Wv.T
  out = softmax(Q K^T / 32) V

Sharding: 8 cores = 4 batches x 2 query-halves. Each core computes K/V for
its batch's full 2048-seq and Q for its 1024-query half; outputs are
disjoint [1024, 1024] slices, so no collectives. (A pairwise-AllGather
K/V-dedup variant was tried and measured SLOWER: 2-rank collective_compute
moves ~34 GB/s, so the 8MB exchange costs ~2x the 55us of PE it saves.)

All inputs are cast to bf16 on the HOST (same PE rate as f32r, half the
DMA and SBUF), and each core's xT is column-rolled so its query half sits
at columns 0..1023 (softmax is permutation-invariant over keys, so K/V on
the rolled order give the identical output). xT then stays resident in
SBUF for all three projections: total DMA is 4MB x + 6MB weights + 4MB
out per core, all prefetched/hidden under matmuls.

Per-core schedule (everything bf16 except PSUM and the final output):
  phase K: KT[e,k] = WkT.T @ xT      -> resident SBUF (4MB), x streamed
           into the resident x_sb in 512-col blocks, wk split per-d so
           the first matmul starts ~1.2us in; wv prefetched mid-phase
  phase V: V[k,e]  = x_sb.T @ WvT    -> resident SBUF (4MB), no DMA;
           wq prefetched at phase start
  phase Q: QT[e,q] = WqT.T @ x_sb[:, 0:1024] -> resident SBUF (2MB), no DMA
  attention per 512-query block, S computed TRANSPOSED so exp output is
  directly the stationary operand of the O matmul (no PE transposes):
      ST[k,q] chunk = KT_slice.T @ QT    per k-tile  [128, 512] PSUM
      attT[k,q]     = exp(ST/32) on ACT  -> SBUF
      O[q,e]        = attT.T @ V         (PSUM accum over k, 2x512 chunks)
      rowsum[q]     = attT.T @ ones      (third 8-wide PSUM group, ~free)
      out = O * (1/rowsum)               -> DRAM
"""

import os
import sys
from contextlib import ExitStack

import numpy as np

for _p in ("/opt/trn_rl_repo", "/root/.axon_site/_ro/trn_rl_repo"):
    if os.path.isdir(_p) and _p not in sys.path:
        sys.path.append(_p)

import concourse.bass as bass
import concourse.mybir as mybir
from concourse import bacc, tile
from concourse.bass_utils import run_bass_kernel_spmd

F32 = mybir.dt.float32
BF16 = mybir.dt.bfloat16

B, SEQ, D = 4, 2048, 1024
QL = SEQ // 2          # queries per core
N_CORES = 8
DT = D // 128          # 8 d-tiles (contraction)
ET = D // 128          # 8 e-tiles (hidden out)
KT = SEQ // 128        # 16 k-tiles
KB = 512               # x-col streaming block width (phase K)
NKB = SEQ // KB        # 4 blocks
QB = 512               # attention q-block width
NQB = QL // QB         # 2 q-blocks
AF = mybir.ActivationFunctionType


def _copy(nc, i, dst, src):
    # alternate PSUM->SBUF copies between DVE and ACT to balance engines
    if i % 2 == 0:
        nc.vector.tensor_copy(dst, src)
    else:
        nc.scalar.copy(dst, src)


def build(reps: int = 1):
    nc = bacc.Bacc(
        "TRN2", target_bir_lowering=False, debug=False, num_devices=N_CORES
    )

    xT = nc.declare_dram_parameter("xT", [D, SEQ], BF16, isOutput=False)
    WqT = nc.declare_dram_parameter("WqT", [D, D], BF16, isOutput=False)
    WkT = nc.declare_dram_parameter("WkT", [D, D], BF16, isOutput=False)
    WvT = nc.declare_dram_parameter("WvT", [D, D], BF16, isOutput=False)
    out = nc.declare_dram_parameter("out", [QL, D], F32, isOutput=True)

    aps = {
        "xT_r": xT.rearrange("(dt p) k -> p dt k", p=128),
        "Wq_r": WqT.rearrange("(dt p) e -> p dt e", p=128),
        "Wk_r": WkT.rearrange("(dt p) e -> p dt e", p=128),
        "Wv_r": WvT.rearrange("(dt p) e -> p dt e", p=128),
        "out_r": out.rearrange("(qt p) e -> qt p e", p=128),
    }

    with ExitStack() as top:
        tc = top.enter_context(tile.TileContext(nc))

        res_pool = top.enter_context(tc.tile_pool(name="res", bufs=1))

        x_sb = res_pool.tile([128, DT, SEQ], BF16, tag="x_sb")
        kt_sb = res_pool.tile([128, ET, SEQ], BF16, tag="kt_sb")
        v_sb = res_pool.tile([128, KT, D], BF16, tag="v_sb")
        qt_sb = res_pool.tile([128, ET, QL], BF16, tag="qt_sb")
        # weights persistent too: cross-rep deps are then per-tile WARs
        # (wk's reload for rep n+1 only waits on rep n's phase-K reads),
        # so the next rep's DMAs prefetch under this rep's attention.
        wk = res_pool.tile([128, DT, D], BF16, tag="wk")
        wv = res_pool.tile([128, DT, D], BF16, tag="wv")
        wq = res_pool.tile([128, DT, D], BF16, tag="wq")
        psp = top.enter_context(tc.tile_pool(name="psp", bufs=2, space="PSUM"))

        for _ in range(reps):
            _body(nc, tc, x_sb, kt_sb, v_sb, qt_sb, wk, wv, wq, psp, aps)

    nc.compile()
    return nc


def _body(nc, tc, x_sb, kt_sb, v_sb, qt_sb, wk, wv, wq, psp, aps):
    xT_r = aps["xT_r"]
    Wq_r = aps["Wq_r"]
    Wk_r = aps["Wk_r"]
    Wv_r = aps["Wv_r"]
    out_r = aps["out_r"]

    # One PSUM pool for the whole build: per-tag rotation pipelines phase
    # boundaries with no release/realloc stall (ps 2 + po0 2 + po1 2 +
    # pos 2 = 8 banks).

    # ---------------- phase K: KT[e,k] resident ----------------
    if True:
        # interleave wk rows with x j=0 rows so the first matmul can start
        # after ~0.6MB of DMA instead of 2.5MB
        for d in range(DT):
            nc.sync.dma_start(wk[:, d, :], Wk_r[:, d, :])
            nc.sync.dma_start(
                x_sb[:, d, 0:KB], xT_r[:, d, 0:KB]
            )
        for j in range(1, NKB):
            nc.sync.dma_start(
                x_sb[:, :, j * KB : (j + 1) * KB],
                xT_r[:, :, j * KB : (j + 1) * KB],
            )

        ci = 0
        for j in range(NKB):
            if j == 1:
                # ~40us of K matmuls remain: hides the 2MB wv load
                nc.sync.dma_start(wv[:], Wv_r[:])
            if j == 0:
                # d-major warm-up over two concurrent PSUM groups so the
                # first matmul needs only wk row 0 + x rows 0
                pss0 = [
                    psp.tile([128, KB], F32, tag="ps", name=f"ps0_{i}")
                    for i in range(2)
                ]
                for d in range(DT):
                    for et in range(2):
                        nc.tensor.matmul(
                            pss0[et][:],
                            wk[:, d, et * 128 : (et + 1) * 128],
                            x_sb[:, d, 0:KB],
                            start=(d == 0),
                            stop=(d == DT - 1),
                        )
                for et in range(2):
                    _copy(nc, ci, kt_sb[:, et, 0:KB], pss0[et][:])
                    ci += 1
                rest = range(2, ET)
            else:
                rest = range(ET)
            for et in rest:
                ps = psp.tile([128, KB], F32, tag="ps")
                for d in range(DT):
                    nc.tensor.matmul(
                        ps[:],
                        wk[:, d, et * 128 : (et + 1) * 128],
                        x_sb[:, d, j * KB : (j + 1) * KB],
                        start=(d == 0),
                        stop=(d == DT - 1),
                    )
                _copy(nc, ci, kt_sb[:, et, j * KB : (j + 1) * KB], ps[:])
                ci += 1

    # ---------------- phase V: V[k,e] resident (no DMA) ----------------
    nc.sync.dma_start(wq[:], Wq_r[:])  # hidden under phase V matmuls

    if True:
        ci = 0
        for kt in range(KT):
            for ec in range(D // 512):
                ps = psp.tile([128, 512], F32, tag="ps")
                for d in range(DT):
                    nc.tensor.matmul(
                        ps[:],
                        x_sb[:, d, kt * 128 : (kt + 1) * 128],
                        wv[:, d, ec * 512 : (ec + 1) * 512],
                        start=(d == 0),
                        stop=(d == DT - 1),
                    )
                _copy(nc, ci, v_sb[:, kt, ec * 512 : (ec + 1) * 512], ps[:])
                ci += 1

    # ------------- phase Q: QT[e,q] resident (no DMA) -------------
    # queries sit at x_sb cols 0..QL-1 (host rolled the columns per-core)
    if True:
        ci = 0
        for qc in range(QL // 512):
            for et in range(ET):
                ps = psp.tile([128, 512], F32, tag="ps")
                for d in range(DT):
                    nc.tensor.matmul(
                        ps[:],
                        wq[:, d, et * 128 : (et + 1) * 128],
                        x_sb[:, d, qc * 512 : (qc + 1) * 512],
                        start=(d == 0),
                        stop=(d == DT - 1),
                    )
                _copy(nc, ci, qt_sb[:, et, qc * 512 : (qc + 1) * 512], ps[:])
                ci += 1

    # ---------------- attention (S transposed) ----------------
    with (
        tc.tile_pool(name="attn", bufs=1) as attn_p,
        tc.tile_pool(name="osb_p", bufs=2) as osb_p,
        tc.tile_pool(name="vec_p", bufs=8) as vec_p,
    ):
        attT = attn_p.tile([128, KT, QB], BF16, tag="attT")
        ones_f = attn_p.tile([128, 8], F32, tag="ones_f")
        ones = attn_p.tile([128, 8], BF16, tag="ones")
        nc.vector.memset(ones_f[:], 1.0)
        nc.vector.tensor_copy(ones[:], ones_f[:])

        for qb in range(NQB):
            for kt in range(KT):
                ps = psp.tile([128, QB], F32, tag="ps")
                for et in range(ET):
                    nc.tensor.matmul(
                        ps[:],
                        kt_sb[:, et, kt * 128 : (kt + 1) * 128],
                        qt_sb[:, et, qb * QB : (qb + 1) * QB],
                        start=(et == 0),
                        stop=(et == ET - 1),
                    )
                nc.scalar.activation(
                    attT[:, kt, :], ps[:], AF.Exp, scale=1.0 / 32.0
                )
            for q4 in range(QB // 128):
                qt = qb * (QB // 128) + q4
                po0 = psp.tile([128, 512], F32, tag="po0")
                po1 = psp.tile([128, 512], F32, tag="po1")
                pos = psp.tile([128, 8], F32, tag="pos")
                pch = (po0, po1)
                att_q = attT[:, :, q4 * 128 : (q4 + 1) * 128]
                for kt in range(KT):
                    st = kt == 0
                    sp = kt == KT - 1
                    for ec in range(2):
                        nc.tensor.matmul(
                            pch[ec][:],
                            att_q[:, kt, :],
                            v_sb[:, kt, ec * 512 : (ec + 1) * 512],
                            start=st,
                            stop=sp,
                        )
                    nc.tensor.matmul(
                        pos[:], att_q[:, kt, :], ones[:], start=st, stop=sp
                    )
                r = vec_p.tile([128, 1], F32, tag="r")
                nc.vector.reciprocal(r[:], pos[:, 0:1])
                osb = osb_p.tile([128, D], F32, tag="osb")
                for ec in range(2):
                    nc.vector.tensor_scalar_mul(
                        osb[:, ec * 512 : (ec + 1) * 512], pch[ec][:], r[:]
                    )
                nc.sync.dma_start(out_r[qt], osb[:])


_CACHE: dict = {}


def _get_nc():
    if "nc" not in _CACHE:
        _CACHE["nc"] = build()
    return _CACHE["nc"]


def _get_runner():
    """Cached jitted shard_map executable over the 8 cores."""
    if "runner" in _CACHE:
        return _CACHE["runner"]

    import jax
    from jax.sharding import Mesh, NamedSharding, PartitionSpec
    from jax.experimental.shard_map import shard_map

    from concourse import bass2jax, mybir as _mybir

    nc = _get_nc()
    bass2jax.install_neuronx_cc_hook()

    partition_name = (
        nc.partition_id_tensor.name if nc.partition_id_tensor else None
    )
    in_names = []
    out_names = []
    out_avals = []
    zero_outs = []
    for alloc in nc.m.functions[0].allocations:
        if not isinstance(alloc, _mybir.MemoryLocationSet):
            continue
        if alloc.kind == "ExternalInput":
            if alloc.memorylocations[0].name == partition_name:
                continue
            in_names.append(alloc.memorylocations[0].name)
        elif alloc.kind == "ExternalOutput":
            name = alloc.memorylocations[0].name
            out_names.append(name)
            shape = tuple(alloc.tensor_shape)
            dtype = _mybir.dt.np(alloc.dtype)
            out_avals.append(jax.core.ShapedArray(shape, dtype))
            zero_outs.append(np.zeros(shape, dtype))
    n_params = len(in_names)
    all_in_names = in_names + out_names
    if partition_name is not None:
        all_in_names = all_in_names + [partition_name]

    def _body_fn(*args):
        operands = list(args)
        if partition_name is not None:
            operands.append(bass2jax.partition_id_tensor())
        outs = bass2jax._bass_exec_p.bind(
            *operands,
            out_avals=tuple(out_avals),
            in_names=tuple(all_in_names),
            out_names=tuple(out_names),
            lowering_input_output_aliases=(),
            sim_require_finite=True,
            sim_require_nnan=True,
            nc=nc,
        )
        return tuple(outs)

    devices = jax.devices()[:N_CORES]
    mesh = Mesh(np.asarray(devices), ("core",))
    spec = PartitionSpec("core")
    n_outs = len(out_names)
    sharded = jax.jit(
        shard_map(
            _body_fn,
            mesh=mesh,
            in_specs=(spec,) * (n_params + n_outs),
            out_specs=(spec,) * n_outs,
            check_rep=False,
        ),
        keep_unused=True,
    )
    sharding = NamedSharding(mesh, spec)

    def run(in_maps):
        concat_in = [
            np.concatenate([np.asarray(m[name]) for m in in_maps], axis=0)
            for name in in_names
        ]
        concat_zeros = [
            np.zeros((N_CORES * z.shape[0], *z.shape[1:]), z.dtype)
            for z in zero_outs
        ]
        dev_in = [jax.device_put(a, sharding) for a in concat_in]
        dev_zero = [jax.device_put(a, sharding) for a in concat_zeros]
        out_arrs = sharded(*dev_in, *dev_zero)
        return [
            {
                name: np.asarray(out_arrs[i]).reshape(
                    N_CORES, *out_avals[i].shape
                )[c]
                for i, name in enumerate(out_names)
            }
            for c in range(N_CORES)
        ]

    def run_device(dev_in, dev_zero):
        return sharded(*dev_in, *dev_zero)

    _CACHE["runner"] = (run, run_device, sharding, in_names, zero_outs)
    return _CACHE["runner"]


def _make_in_maps(x, Wq, Wk, Wv):
    bf16 = mybir.dt.np(BF16)
    x = np.asarray(x, dtype=np.float32)
    wqT = np.ascontiguousarray(np.asarray(Wq, dtype=np.float32).T).astype(bf16)
    wkT = np.ascontiguousarray(np.asarray(Wk, dtype=np.float32).T).astype(bf16)
    wvT = np.ascontiguousarray(np.asarray(Wv, dtype=np.float32).T).astype(bf16)
    in_maps = []
    for c in range(N_CORES):
        b, h = divmod(c, 2)
        # roll columns so this core's query half sits at cols 0..QL-1;
        # K/V see a permuted key order, which softmax-attention is
        # invariant to.
        xb = np.ascontiguousarray(
            np.roll(x[b].T, -h * QL, axis=1)
        ).astype(bf16)
        in_maps.append(
            {
                "xT": xb,
                "WqT": wqT,
                "WkT": wkT,
                "WvT": wvT,
            }
        )
    return in_maps


def _assemble(results):
    out = np.empty((B, SEQ, D), dtype=np.float32)
    for c in range(N_CORES):
        b, h = divmod(c, 2)
        out[b, h * QL : (h + 1) * QL] = results[c]["out"]
    return out


def run_traced(x, Wq, Wk, Wv, **kw):
    """Run via run_bass_kernel_spmd, return (output, BassKernelResults)."""
    nc = _get_nc()
    res = run_bass_kernel_spmd(
        nc, _make_in_maps(x, Wq, Wk, Wv), list(range(N_CORES)), **kw
    )
    return _assemble(res.results), res


def kernel(x, Wq, Wk, Wv):
    # The axon tunnel occasionally throws transient device errors
    # (worker hung up / mesh desynced / NRT_EXEC_UNIT_UNRECOVERABLE) that
    # clear on retry; rebuild the jitted runner on the second attempt.
    in_maps = _make_in_maps(x, Wq, Wk, Wv)
    last_err = None
    for attempt in range(3):
        try:
            run, _, _, _, _ = _get_runner()
            return _assemble(run(in_maps))
        except Exception as e:  # noqa: BLE001 - retry transient device faults
            last_err = e
            import time as _time

            _time.sleep(5 * (attempt + 1))
            _CACHE.pop("runner", None)
    raise last_err
